# revision 5
# baseline (speedup 1.0000x reference)
"""KMeans vq_codebook kernel v2 for 8 trn2 NeuronCores.

Data-parallel over N (32768 rows/core, 256 row-tiles of 128). Per tile:
  PE:   g2 = xh @ ch (fp16, fp32 PSUM) ; batched csq fold g2 += 1*(128-csq)
        so PSUM holds h = 2x.c - csq + 128
  ACT:  h16 = fp16(h) -> SBUF (batched over 4-tile PSUM groups)
  DVE:  hierarchical max (tt-max halves at 2x + final reduce) -> hmax f32
  DVE/Pool (tile-routed 1:3): onehot oh = (h16 >= hmax) as fp8
  PE:   conf[10,256] += DoubleRow fp8 matmul of (yoh pair, oh pair)
Host: loss = sum(x^2) + 128N - sum(hmax); acc from summed conf.
"""

import sys

sys.path.insert(0, "/opt/trn_rl_repo")

import numpy as np

import concourse.bass as bass
import concourse.mybir as mybir
import concourse.tile as tile
from concourse.bass_utils import run_bass_kernel_spmd

N_FULL = 262144
D = 128
K = 256
NUM_CORES = 8
NS = N_FULL // NUM_CORES  # 32768 rows per core
NTILES = NS // 128  # 256
NGC = 10
NGCP = 16  # padded class dim: DoubleRow ldweights needs 16B-aligned pair stride
OFF = 128.0

F32 = mybir.dt.float32
F16 = mybir.dt.float16
F8 = mybir.dt.float8e4

_CACHE = {}


def build_nc(
    supertile=2048,
    dve_every=999,
    dve_copy_every=8,
    pool_l1_every=999,
    psbufs=3,
    hbufs=5,
    xbufs=3,
    nd4=0,
    for_sim=False,
):
    ntiles = NTILES

    import concourse.bacc as bacc

    nc = bacc.Bacc("TRN2", target_bir_lowering=False, debug=bool(for_sim))

    xh_d = nc.declare_dram_parameter("xh", [D, NS], F16, isOutput=False)
    ch_d = nc.declare_dram_parameter("ch", [D, K], F16, isOutput=False)
    csqm2_d = nc.declare_dram_parameter("csqm2", [1, 2 * K], F16, isOutput=False)
    csqs_d = nc.declare_dram_parameter("csqs32", [128, K], F32, isOutput=False)
    yohp_d = nc.declare_dram_parameter(
        "yohp", [128, ntiles // 2, 2, NGCP], F8, isOutput=False
    )
    hmax_out = nc.declare_dram_parameter("hmax", [128, ntiles], F32, isOutput=True)
    conf_out = nc.declare_dram_parameter("conf", [NGCP, K], F32, isOutput=True)

    with tile.TileContext(nc) as tc:
        with (
            tc.tile_pool(name="const", bufs=1) as constp,
            tc.tile_pool(name="xs", bufs=xbufs) as xsp,
            tc.tile_pool(name="hg", bufs=hbufs) as hgp,
            tc.tile_pool(name="mx", bufs=hbufs) as mxp,
            tc.tile_pool(name="ohp", bufs=hbufs) as ohp,
            tc.tile_pool(name="acc", bufs=1) as accp,
            tc.tile_pool(name="ps", bufs=psbufs, space=bass.MemorySpace.PSUM) as psp,
            tc.tile_pool(name="psconf", bufs=1, space=bass.MemorySpace.PSUM) as pscp,
        ):
            ch_t = constp.tile([D, K], F16, tag="ch")
            csqm2_t = constp.tile([1, 2 * K], F16, tag="csqm2")
            csqs_t = constp.tile([128, K], F32, tag="csqs32")
            yoh_t = constp.tile([128, ntiles // 2, 2, NGCP], F8, tag="yoh")
            ones_t = constp.tile([1, D], F16, tag="ones")
            # ch on SP first (PE needs it immediately); the rest via the idle
            # Pool queue so they don't delay the first xh chunk on SP
            nc.sync.dma_start(ch_t[:], ch_d[:, :])
            nc.gpsimd.dma_start(csqm2_t[:], csqm2_d[:, :])
            nc.gpsimd.dma_start(csqs_t[:], csqs_d[:, :])
            nc.gpsimd.dma_start(yoh_t[:], yohp_d[:, :, :, :])
            nc.vector.memset(ones_t[:], 1.0)

            hmax_acc = accp.tile([128, ntiles], F32, tag="hmax")
            conf_ps = pscp.tile([NGCP, K], F32, tag="conf")

            # chunks of columns: small first chunk so compute starts early
            chunks = [1024, 1024] + [supertile] * ((NS - 2048) // supertile)
            assert sum(chunks) == NS

            state = {"first_conf": True}
            pending_conf = []  # deferred conf-matmul closures (1-group delay)

            def emit_conf(g, oh):
                def emit():
                    for p in range(4):
                        pj = g * 4 + p  # global pair id
                        nc.tensor.matmul(
                            conf_ps[:],
                            yoh_t[:, pj, :, :],
                            oh[:, 2 * p : 2 * p + 2, :],
                            start=state["first_conf"],
                            stop=(pj == ntiles // 2 - 1),
                            perf_mode=mybir.MatmulPerfMode.DoubleRow,
                            skip_group_check=True,
                        )
                        state["first_conf"] = False

                return emit

            col0 = 0
            g = -1
            for st, width in enumerate(chunks):
                xh_s = xsp.tile([D, width], F16, tag=f"xh{width}")
                if st == 0:
                    # split first chunk's DMA so PE can start sooner
                    nc.sync.dma_start(
                        xh_s[:, 0 : width // 2], xh_d[:, col0 : col0 + width // 2]
                    )
                    nc.sync.dma_start(
                        xh_s[:, width // 2 : width],
                        xh_d[:, col0 + width // 2 : col0 + width],
                    )
                else:
                    nc.sync.dma_start(xh_s[:], xh_d[:, col0 : col0 + width])
                tps = width // 128
                for gg in range(tps // 8):  # sbuf groups of 8 tiles
                    g += 1  # global sbuf-group id
                    h16 = hgp.tile([128, 8, K], F16, tag="h16")
                    for pg in range(2):  # psum groups of 4 tiles
                        pgid = g * 2 + pg  # global psum-group id
                        dve_copy = pgid % dve_copy_every == dve_copy_every - 1
                        g2 = psp.tile([128, 4, K], F32, tag="g2")
                        for q in range(4):
                            t_local = gg * 8 + pg * 4 + q  # tile in supertile
                            sl = slice(t_local * 128, (t_local + 1) * 128)
                            # start=True zeroes the whole 2KB PSUM bank (2
                            # tiles); only the first matmul per bank sets it.
                            # PE runs in emission order, so q, q+1 are safe.
                            nc.tensor.matmul(
                                g2[:, q, :],
                                xh_s[:, sl],
                                ch_t[:],
                                start=(q % 2 == 0),
                                stop=(dve_copy and q == 3),
                                skip_group_check=True,
                            )
                        if dve_copy:
                            # DVE: h16 = g2 - (csq - 128), csq folded here
                            csqb = csqs_t[:].unsqueeze(1).broadcast_to([128, 4, K])
                            nc.vector.tensor_tensor(
                                out=h16[:, pg * 4 : pg * 4 + 4, :],
                                in0=g2[:],
                                in1=csqb,
                                op=mybir.AluOpType.subtract,
                            )
                        else:
                            # csq fold on PE: adds (128 - csq), 2 tiles per
                            # matmul (PSUM bank limit: output <= 2 KB/bank)
                            nc.tensor.matmul(
                                g2[:, 0:2, :],
                                ones_t[:],
                                csqm2_t[:],
                                start=False,
                                stop=False,
                                skip_group_check=True,
                            )
                            nc.tensor.matmul(
                                g2[:, 2:4, :],
                                ones_t[:],
                                csqm2_t[:],
                                start=False,
                                stop=True,
                                skip_group_check=True,
                            )
                            nc.scalar.activation(
                                out=h16[:, pg * 4 : pg * 4 + 4, :],
                                in_=g2[:],
                                func=mybir.ActivationFunctionType.Copy,
                            )
                    # hierarchical max over K within the 8-tile group
                    m1 = mxp.tile([128, 8, K // 2], F16, tag="m1")
                    m2 = mxp.tile([128, 8, K // 4], F16, tag="m2")
                    l1_eng = (
                        nc.gpsimd
                        if g % pool_l1_every == pool_l1_every - 1
                        else nc.vector
                    )
                    l1_eng.tensor_tensor(
                        out=m1[:],
                        in0=h16[:, :, 0 : K // 2],
                        in1=h16[:, :, K // 2 : K],
                        op=mybir.AluOpType.max,
                    )
                    nc.vector.tensor_tensor(
                        out=m2[:],
                        in0=m1[:, :, 0 : K // 4],
                        in1=m1[:, :, K // 4 : K // 2],
                        op=mybir.AluOpType.max,
                    )
                    nc.vector.tensor_reduce(
                        hmax_acc[:, g * 8 : g * 8 + 8],
                        m2[:],
                        axis=mybir.AxisListType.X,
                        op=mybir.AluOpType.max,
                    )
                    # onehot, tile-routed between DVE (1 in dve_every) and Pool;
                    # last groups split across both engines to drain fast
                    n_groups = ntiles // 8
                    oh = ohp.tile([128, 8, K], F8, tag="oh")

                    def oh_dve(q0, q1):
                        for q in range(q0, q1):
                            j = g * 8 + q
                            nc.vector.tensor_scalar(
                                out=oh[:, q, :],
                                in0=h16[:, q, :],
                                scalar1=hmax_acc[:, j : j + 1],
                                scalar2=None,
                                op0=mybir.AluOpType.is_ge,
                            )

                    def oh_pool(q0, q1):
                        # walrus rejects TensorTensor on Pool; per-tile
                        # tensor_scalar (TensorScalarPtr) is the valid form
                        for q in range(q0, q1):
                            j = g * 8 + q
                            nc.gpsimd.tensor_scalar(
                                out=oh[:, q, :],
                                in0=h16[:, q, :],
                                scalar1=hmax_acc[:, j : j + 1],
                                scalar2=None,
                                op0=mybir.AluOpType.is_ge,
                            )

                    if g >= n_groups - 2:
                        oh_pool(0, 4)
                        oh_dve(4, 8)
                    elif g % dve_every == dve_every - 1:
                        oh_dve(0, 8)
                    else:
                        # steady state: nd4/4 tiles per group on DVE, rest Pool
                        n_d = nd4 // 4 + (1 if g % 4 < nd4 % 4 else 0)
                        oh_dve(0, n_d)
                        oh_pool(n_d, 8)
                    # conf accumulation deferred by one group so the in-order
                    # PE queue never stalls waiting for this group's onehot
                    pending_conf.append(emit_conf(g, oh))
                    if len(pending_conf) > 1:
                        pending_conf.pop(0)()
                    # drain hmax output as it completes
                    if g % 8 == 7:
                        nc.sync.dma_start(
                            hmax_out[:, (g - 7) * 8 : (g + 1) * 8],
                            hmax_acc[:, (g - 7) * 8 : (g + 1) * 8],
                        )
                col0 += width

            for emit in pending_conf:
                emit()

            conf_sb = accp.tile([NGCP, K], F32, tag="confsb")
            nc.vector.tensor_copy(conf_sb[:], conf_ps[:])
            nc.sync.dma_start(conf_out[:, :], conf_sb[:])

    nc.compile()
    return nc


def kernel(x, y, centers):
    x = np.asarray(x, dtype=np.float32)
    y_np = np.asarray(y).astype(np.int64)
    centers = np.asarray(centers, dtype=np.float32)
    n = x.shape[0]
    assert n == N_FULL and x.shape[1] == D and centers.shape == (K, D)

    if "nc" not in _CACHE:
        _CACHE["nc"] = build_nc()
    nc = _CACHE["nc"]

    f8np = mybir.dt.np(F8)

    xt = np.ascontiguousarray(x.T).astype(np.float16)  # [128, N]
    ch = np.ascontiguousarray(centers.T * np.float32(2.0)).astype(np.float16)
    csq = np.sum(centers.astype(np.float64) ** 2, axis=1)
    csqm = (np.float64(OFF) - csq).astype(np.float16)  # [K]
    csqm2 = np.ascontiguousarray(np.tile(csqm, 2)[None, :])  # [1, 2K]
    csqs32 = np.ascontiguousarray(
        np.broadcast_to((csq - np.float64(OFF)).astype(np.float32)[None, :], (128, K))
    )

    # yoh pairs: [core, 128, npairs, 2, 10] in fp8
    y_cores = y_np.reshape(NUM_CORES, NTILES, 128)
    oh = y_cores[:, :, :, None] == np.arange(NGCP)[None, None, None, :]
    # [core, tile, p, c] -> [core, p(128), pair, j, c]  (classes 10..15 never
    # match y, so the padded conf rows stay zero)
    oh = oh.reshape(NUM_CORES, NTILES // 2, 2, 128, NGCP).transpose(0, 3, 1, 2, 4)
    yohp_all = np.ascontiguousarray(
        oh.reshape(NUM_CORES, 128, (NTILES // 2) * 2 * NGCP)
    ).astype(f8np)

    in_maps = []
    for c in range(NUM_CORES):
        sl = slice(c * NS, (c + 1) * NS)
        in_maps.append(
            {
                "xh": np.ascontiguousarray(xt[:, sl]),
                "ch": ch,
                "csqm2": csqm2,
                "csqs32": csqs32,
                "yohp": yohp_all[c],
            }
        )

    full_res = run_bass_kernel_spmd(nc, in_maps, list(range(NUM_CORES)))
    _CACHE["last_results"] = full_res
    res = full_res.results

    hmax_sum = 0.0
    conf = np.zeros((K, NGC), dtype=np.float64)
    for c in range(NUM_CORES):
        hmax_sum += float(np.asarray(res[c]["hmax"]).astype(np.float64).sum())
        conf += np.asarray(res[c]["conf"]).astype(np.float64).T[:, :NGC]  # [K, 10]

    x64 = x.astype(np.float64)
    x_sq_total = float(np.einsum("nd,nd->", x64, x64, optimize=True))
    loss = np.float32(x_sq_total + OFF * n - hmax_sum)

    correct_ct = conf.max(axis=1).sum()
    acc = np.float32(correct_ct / np.float32(n))
    return loss, acc


# revision 6
# speedup vs baseline: 1.0057x; 1.0057x over previous
"""KMeans vq_codebook kernel v2 for 8 trn2 NeuronCores.

Data-parallel over N (32768 rows/core, 256 row-tiles of 128). Per tile:
  PE:   g2 = xh @ ch (fp16, fp32 PSUM) ; batched csq fold g2 += 1*(128-csq)
        so PSUM holds h = 2x.c - csq + 128
  ACT:  h16 = fp16(h) -> SBUF (batched over 4-tile PSUM groups)
  DVE:  hierarchical max (tt-max halves at 2x + final reduce) -> hmax f32
  DVE/Pool (tile-routed 1:3): onehot oh = (h16 >= hmax) as fp8
  PE:   conf[10,256] += DoubleRow fp8 matmul of (yoh pair, oh pair)
Host: loss = sum(x^2) + 128N - sum(hmax); acc from summed conf.
"""

import sys

sys.path.insert(0, "/opt/trn_rl_repo")

import numpy as np

import concourse.bass as bass
import concourse.mybir as mybir
import concourse.tile as tile
from concourse.bass_utils import run_bass_kernel_spmd

N_FULL = 262144
D = 128
K = 256
NUM_CORES = 8
NS = N_FULL // NUM_CORES  # 32768 rows per core
NTILES = NS // 128  # 256
NGC = 10
NGCP = 16  # padded class dim: DoubleRow ldweights needs 16B-aligned pair stride
OFF = 128.0

F32 = mybir.dt.float32
F16 = mybir.dt.float16
F8 = mybir.dt.float8e4

_CACHE = {}


def build_nc(
    supertile=2048,
    dve_every=999,
    dve_copy_every=8,
    pool_l1_every=999,
    psbufs=3,
    hbufs=5,
    xbufs=3,
    nd4=0,
    conf_defer=1,
    for_sim=False,
):
    ntiles = NTILES

    import concourse.bacc as bacc

    nc = bacc.Bacc("TRN2", target_bir_lowering=False, debug=bool(for_sim))

    xh_d = nc.declare_dram_parameter("xh", [D, NS], F16, isOutput=False)
    ch_d = nc.declare_dram_parameter("ch", [D, K], F16, isOutput=False)
    csqm2_d = nc.declare_dram_parameter("csqm2", [1, 2 * K], F16, isOutput=False)
    csqs_d = nc.declare_dram_parameter("csqs32", [128, K], F32, isOutput=False)
    yohp_d = nc.declare_dram_parameter(
        "yohp", [128, ntiles // 2, 2, NGCP], F8, isOutput=False
    )
    hmax_out = nc.declare_dram_parameter("hmax", [128, ntiles], F32, isOutput=True)
    conf_out = nc.declare_dram_parameter("conf", [NGCP, K], F32, isOutput=True)

    with tile.TileContext(nc) as tc:
        with (
            tc.tile_pool(name="const", bufs=1) as constp,
            tc.tile_pool(name="xs", bufs=xbufs) as xsp,
            tc.tile_pool(name="hg", bufs=hbufs) as hgp,
            tc.tile_pool(name="mx", bufs=hbufs) as mxp,
            tc.tile_pool(name="ohp", bufs=hbufs) as ohp,
            tc.tile_pool(name="acc", bufs=1) as accp,
            tc.tile_pool(name="ps", bufs=psbufs, space=bass.MemorySpace.PSUM) as psp,
            tc.tile_pool(name="psconf", bufs=1, space=bass.MemorySpace.PSUM) as pscp,
        ):
            ch_t = constp.tile([D, K], F16, tag="ch")
            csqm2_t = constp.tile([1, 2 * K], F16, tag="csqm2")
            csqs_t = constp.tile([128, K], F32, tag="csqs32")
            yoh_t = constp.tile([128, ntiles // 2, 2, NGCP], F8, tag="yoh")
            ones_t = constp.tile([1, D], F16, tag="ones")
            # consts via the idle Pool queue (cheap dispatch) so the first xh
            # chunk is issued on SP immediately; ch first (PE needs it first)
            nc.gpsimd.dma_start(ch_t[:], ch_d[:, :])
            nc.gpsimd.dma_start(csqm2_t[:], csqm2_d[:, :])
            nc.gpsimd.dma_start(csqs_t[:], csqs_d[:, :])
            nc.gpsimd.dma_start(yoh_t[:], yohp_d[:, :, :, :])
            nc.vector.memset(ones_t[:], 1.0)

            hmax_acc = accp.tile([128, ntiles], F32, tag="hmax")
            conf_ps = pscp.tile([NGCP, K], F32, tag="conf")

            # chunks of columns: small first chunk so compute starts early
            chunks = [1024, 1024] + [supertile] * ((NS - 2048) // supertile)
            assert sum(chunks) == NS

            state = {"first_conf": True}
            pending_conf = []  # deferred conf-matmul closures (1-group delay)

            def emit_conf(g, oh):
                def emit():
                    for p in range(4):
                        pj = g * 4 + p  # global pair id
                        nc.tensor.matmul(
                            conf_ps[:],
                            yoh_t[:, pj, :, :],
                            oh[:, 2 * p : 2 * p + 2, :],
                            start=state["first_conf"],
                            stop=(pj == ntiles // 2 - 1),
                            perf_mode=mybir.MatmulPerfMode.DoubleRow,
                            skip_group_check=True,
                        )
                        state["first_conf"] = False

                return emit

            col0 = 0
            g = -1
            for st, width in enumerate(chunks):
                xh_s = xsp.tile([D, width], F16, tag=f"xh{width}")
                if st == 0:
                    # split first chunk's DMA so PE can start sooner
                    nc.sync.dma_start(
                        xh_s[:, 0 : width // 2], xh_d[:, col0 : col0 + width // 2]
                    )
                    nc.sync.dma_start(
                        xh_s[:, width // 2 : width],
                        xh_d[:, col0 + width // 2 : col0 + width],
                    )
                else:
                    nc.sync.dma_start(xh_s[:], xh_d[:, col0 : col0 + width])
                tps = width // 128
                for gg in range(tps // 8):  # sbuf groups of 8 tiles
                    g += 1  # global sbuf-group id
                    h16 = hgp.tile([128, 8, K], F16, tag="h16")
                    for pg in range(2):  # psum groups of 4 tiles
                        pgid = g * 2 + pg  # global psum-group id
                        dve_copy = pgid % dve_copy_every == dve_copy_every - 1
                        g2 = psp.tile([128, 4, K], F32, tag="g2")
                        for q in range(4):
                            t_local = gg * 8 + pg * 4 + q  # tile in supertile
                            sl = slice(t_local * 128, (t_local + 1) * 128)
                            # start=True zeroes the whole 2KB PSUM bank (2
                            # tiles); only the first matmul per bank sets it.
                            # PE runs in emission order, so q, q+1 are safe.
                            nc.tensor.matmul(
                                g2[:, q, :],
                                xh_s[:, sl],
                                ch_t[:],
                                start=(q % 2 == 0),
                                stop=(dve_copy and q == 3),
                                skip_group_check=True,
                            )
                        if dve_copy:
                            # DVE: h16 = g2 - (csq - 128), csq folded here
                            csqb = csqs_t[:].unsqueeze(1).broadcast_to([128, 4, K])
                            nc.vector.tensor_tensor(
                                out=h16[:, pg * 4 : pg * 4 + 4, :],
                                in0=g2[:],
                                in1=csqb,
                                op=mybir.AluOpType.subtract,
                            )
                        else:
                            # csq fold on PE: adds (128 - csq), 2 tiles per
                            # matmul (PSUM bank limit: output <= 2 KB/bank)
                            nc.tensor.matmul(
                                g2[:, 0:2, :],
                                ones_t[:],
                                csqm2_t[:],
                                start=False,
                                stop=False,
                                skip_group_check=True,
                            )
                            nc.tensor.matmul(
                                g2[:, 2:4, :],
                                ones_t[:],
                                csqm2_t[:],
                                start=False,
                                stop=True,
                                skip_group_check=True,
                            )
                            nc.scalar.activation(
                                out=h16[:, pg * 4 : pg * 4 + 4, :],
                                in_=g2[:],
                                func=mybir.ActivationFunctionType.Copy,
                            )
                    # hierarchical max over K within the 8-tile group
                    m1 = mxp.tile([128, 8, K // 2], F16, tag="m1")
                    m2 = mxp.tile([128, 8, K // 4], F16, tag="m2")
                    l1_eng = (
                        nc.gpsimd
                        if g % pool_l1_every == pool_l1_every - 1
                        else nc.vector
                    )
                    l1_eng.tensor_tensor(
                        out=m1[:],
                        in0=h16[:, :, 0 : K // 2],
                        in1=h16[:, :, K // 2 : K],
                        op=mybir.AluOpType.max,
                    )
                    nc.vector.tensor_tensor(
                        out=m2[:],
                        in0=m1[:, :, 0 : K // 4],
                        in1=m1[:, :, K // 4 : K // 2],
                        op=mybir.AluOpType.max,
                    )
                    nc.vector.tensor_reduce(
                        hmax_acc[:, g * 8 : g * 8 + 8],
                        m2[:],
                        axis=mybir.AxisListType.X,
                        op=mybir.AluOpType.max,
                    )
                    # onehot, tile-routed between DVE (1 in dve_every) and Pool;
                    # last groups split across both engines to drain fast
                    n_groups = ntiles // 8
                    oh = ohp.tile([128, 8, K], F8, tag="oh")

                    def oh_dve(q0, q1):
                        for q in range(q0, q1):
                            j = g * 8 + q
                            nc.vector.tensor_scalar(
                                out=oh[:, q, :],
                                in0=h16[:, q, :],
                                scalar1=hmax_acc[:, j : j + 1],
                                scalar2=None,
                                op0=mybir.AluOpType.is_ge,
                            )

                    def oh_pool(q0, q1):
                        # walrus rejects TensorTensor on Pool; per-tile
                        # tensor_scalar (TensorScalarPtr) is the valid form
                        for q in range(q0, q1):
                            j = g * 8 + q
                            nc.gpsimd.tensor_scalar(
                                out=oh[:, q, :],
                                in0=h16[:, q, :],
                                scalar1=hmax_acc[:, j : j + 1],
                                scalar2=None,
                                op0=mybir.AluOpType.is_ge,
                            )

                    if g >= n_groups - 2:
                        oh_pool(0, 4)
                        oh_dve(4, 8)
                    elif g % dve_every == dve_every - 1:
                        oh_dve(0, 8)
                    else:
                        # steady state: nd4/4 tiles per group on DVE, rest Pool
                        n_d = nd4 // 4 + (1 if g % 4 < nd4 % 4 else 0)
                        oh_dve(0, n_d)
                        oh_pool(n_d, 8)
                    # conf accumulation deferred by one group so the in-order
                    # PE queue never stalls waiting for this group's onehot
                    pending_conf.append(emit_conf(g, oh))
                    if len(pending_conf) > conf_defer:
                        pending_conf.pop(0)()
                    # drain hmax output as it completes (Pool queue: 25 ns
                    # dispatch vs 500+ on SP, matters for the last chunk)
                    if g % 8 == 7:
                        nc.gpsimd.dma_start(
                            hmax_out[:, (g - 7) * 8 : (g + 1) * 8],
                            hmax_acc[:, (g - 7) * 8 : (g + 1) * 8],
                        )
                col0 += width

            for emit in pending_conf:
                emit()

            conf_sb = accp.tile([NGCP, K], F32, tag="confsb")
            nc.vector.tensor_copy(conf_sb[:], conf_ps[:])
            nc.gpsimd.dma_start(conf_out[:, :], conf_sb[:])

    nc.compile()
    return nc


def kernel(x, y, centers):
    x = np.asarray(x, dtype=np.float32)
    y_np = np.asarray(y).astype(np.int64)
    centers = np.asarray(centers, dtype=np.float32)
    n = x.shape[0]
    assert n == N_FULL and x.shape[1] == D and centers.shape == (K, D)

    if "nc" not in _CACHE:
        _CACHE["nc"] = build_nc()
    nc = _CACHE["nc"]

    f8np = mybir.dt.np(F8)

    xt = np.ascontiguousarray(x.T).astype(np.float16)  # [128, N]
    ch = np.ascontiguousarray(centers.T * np.float32(2.0)).astype(np.float16)
    csq = np.sum(centers.astype(np.float64) ** 2, axis=1)
    csqm = (np.float64(OFF) - csq).astype(np.float16)  # [K]
    csqm2 = np.ascontiguousarray(np.tile(csqm, 2)[None, :])  # [1, 2K]
    csqs32 = np.ascontiguousarray(
        np.broadcast_to((csq - np.float64(OFF)).astype(np.float32)[None, :], (128, K))
    )

    # yoh pairs: [core, 128, npairs, 2, 10] in fp8
    y_cores = y_np.reshape(NUM_CORES, NTILES, 128)
    oh = y_cores[:, :, :, None] == np.arange(NGCP)[None, None, None, :]
    # [core, tile, p, c] -> [core, p(128), pair, j, c]  (classes 10..15 never
    # match y, so the padded conf rows stay zero)
    oh = oh.reshape(NUM_CORES, NTILES // 2, 2, 128, NGCP).transpose(0, 3, 1, 2, 4)
    yohp_all = np.ascontiguousarray(
        oh.reshape(NUM_CORES, 128, (NTILES // 2) * 2 * NGCP)
    ).astype(f8np)

    in_maps = []
    for c in range(NUM_CORES):
        sl = slice(c * NS, (c + 1) * NS)
        in_maps.append(
            {
                "xh": np.ascontiguousarray(xt[:, sl]),
                "ch": ch,
                "csqm2": csqm2,
                "csqs32": csqs32,
                "yohp": yohp_all[c],
            }
        )

    full_res = run_bass_kernel_spmd(nc, in_maps, list(range(NUM_CORES)))
    _CACHE["last_results"] = full_res
    res = full_res.results

    hmax_sum = 0.0
    conf = np.zeros((K, NGC), dtype=np.float64)
    for c in range(NUM_CORES):
        hmax_sum += float(np.asarray(res[c]["hmax"]).astype(np.float64).sum())
        conf += np.asarray(res[c]["conf"]).astype(np.float64).T[:, :NGC]  # [K, 10]

    x64 = x.astype(np.float64)
    x_sq_total = float(np.einsum("nd,nd->", x64, x64, optimize=True))
    loss = np.float32(x_sq_total + OFF * n - hmax_sum)

    correct_ct = conf.max(axis=1).sum()
    acc = np.float32(correct_ct / np.float32(n))
    return loss, acc


# revision 7
# speedup vs baseline: 1.0204x; 1.0147x over previous
"""KMeans vq_codebook kernel v2 for 8 trn2 NeuronCores.

Data-parallel over N (32768 rows/core, 256 row-tiles of 128). Per tile:
  PE:   g2 = xh @ ch (fp16, fp32 PSUM) ; batched csq fold g2 += 1*(128-csq)
        so PSUM holds h = 2x.c - csq + 128
  ACT:  h16 = fp16(h) -> SBUF (batched over 4-tile PSUM groups)
  DVE:  hierarchical max (tt-max halves at 2x + final reduce) -> hmax f32
  DVE/Pool (tile-routed 1:3): onehot oh = (h16 >= hmax) as fp8
  PE:   conf[10,256] += DoubleRow fp8 matmul of (yoh pair, oh pair)
Host: loss = sum(x^2) + 128N - sum(hmax); acc from summed conf.
"""

import sys

sys.path.insert(0, "/opt/trn_rl_repo")

import numpy as np

import concourse.bass as bass
import concourse.mybir as mybir
import concourse.tile as tile
from concourse.bass_utils import run_bass_kernel_spmd

N_FULL = 262144
D = 128
K = 256
NUM_CORES = 8
NS = N_FULL // NUM_CORES  # 32768 rows per core
NTILES = NS // 128  # 256
NGC = 10
NGCP = 16  # padded class dim: DoubleRow ldweights needs 16B-aligned pair stride
OFF = 128.0

F32 = mybir.dt.float32
F16 = mybir.dt.float16
F8 = mybir.dt.float8e4

_CACHE = {}


def build_nc(
    supertile=2048,
    dve_every=999,
    dve_copy_every=8,
    pool_l1_every=999,
    psbufs=3,
    hbufs=5,
    xbufs=3,
    nd4=0,
    conf_defer=1,
    for_sim=False,
):
    ntiles = NTILES

    import concourse.bacc as bacc

    nc = bacc.Bacc("TRN2", target_bir_lowering=False, debug=bool(for_sim))

    xh_d = nc.declare_dram_parameter("xh", [D, NS], F16, isOutput=False)
    ch_d = nc.declare_dram_parameter("ch", [D, K], F16, isOutput=False)
    csqm2_d = nc.declare_dram_parameter("csqm2", [1, 2 * K], F16, isOutput=False)
    csqs_d = nc.declare_dram_parameter("csqs32", [128, K], F32, isOutput=False)
    yohp_d = nc.declare_dram_parameter(
        "yohp", [128, ntiles // 2, 2, NGCP], F8, isOutput=False
    )
    hmax_out = nc.declare_dram_parameter("hmax", [128, ntiles], F32, isOutput=True)
    conf_out = nc.declare_dram_parameter("conf", [NGCP, K], F32, isOutput=True)

    with tile.TileContext(nc) as tc:
        with (
            tc.tile_pool(name="const", bufs=1) as constp,
            tc.tile_pool(name="xs", bufs=xbufs) as xsp,
            tc.tile_pool(name="hg", bufs=hbufs) as hgp,
            tc.tile_pool(name="mx", bufs=hbufs) as mxp,
            tc.tile_pool(name="ohp", bufs=hbufs) as ohp,
            tc.tile_pool(name="acc", bufs=1) as accp,
            tc.tile_pool(name="ps", bufs=psbufs, space=bass.MemorySpace.PSUM) as psp,
            tc.tile_pool(name="psconf", bufs=1, space=bass.MemorySpace.PSUM) as pscp,
        ):
            ch_t = constp.tile([D, K], F16, tag="ch")
            csqm2_t = constp.tile([1, 2 * K], F16, tag="csqm2")
            csqs_t = constp.tile([128, K], F32, tag="csqs32")
            yoh_t = constp.tile([128, ntiles // 2, 2, NGCP], F8, tag="yoh")
            ones_t = constp.tile([1, D], F16, tag="ones")
            # consts via the idle Pool queue (cheap dispatch) so the first xh
            # chunk is issued on SP immediately; ch first (PE needs it first)
            nc.gpsimd.dma_start(ch_t[:], ch_d[:, :])
            nc.gpsimd.dma_start(csqm2_t[:], csqm2_d[:, :])
            nc.gpsimd.dma_start(csqs_t[:], csqs_d[:, :])
            nc.gpsimd.dma_start(yoh_t[:], yohp_d[:, :, :, :])
            nc.vector.memset(ones_t[:], 1.0)

            hmax_acc = accp.tile([128, ntiles], F32, tag="hmax")
            conf_ps = pscp.tile([NGCP, K], F32, tag="conf")

            # chunks of columns: small first chunk so compute starts early
            chunks = [1024, 1024] + [supertile] * ((NS - 2048) // supertile)
            assert sum(chunks) == NS

            state = {"first_conf": True}
            pending_conf = []  # deferred conf-matmul closures (1-group delay)

            def emit_conf(g, oh):
                def emit():
                    for p in range(4):
                        pj = g * 4 + p  # global pair id
                        nc.tensor.matmul(
                            conf_ps[:],
                            yoh_t[:, pj, :, :],
                            oh[:, 2 * p : 2 * p + 2, :],
                            start=state["first_conf"],
                            stop=(pj == ntiles // 2 - 1),
                            perf_mode=mybir.MatmulPerfMode.DoubleRow,
                            skip_group_check=True,
                        )
                        state["first_conf"] = False

                return emit

            col0 = 0
            g = -1
            for st, width in enumerate(chunks):
                xh_s = xsp.tile([D, width], F16, tag=f"xh{width}")
                if st == 0:
                    # split first chunk's DMA so PE can start sooner
                    nc.sync.dma_start(
                        xh_s[:, 0 : width // 2], xh_d[:, col0 : col0 + width // 2]
                    )
                    nc.sync.dma_start(
                        xh_s[:, width // 2 : width],
                        xh_d[:, col0 + width // 2 : col0 + width],
                    )
                else:
                    nc.sync.dma_start(xh_s[:], xh_d[:, col0 : col0 + width])
                tps = width // 128
                for gg in range(tps // 8):  # sbuf groups of 8 tiles
                    g += 1  # global sbuf-group id
                    h16 = hgp.tile([128, 8, K], F16, tag="h16")
                    for pg in range(2):  # psum groups of 4 tiles
                        pgid = g * 2 + pg  # global psum-group id
                        dve_copy = pgid % dve_copy_every == 0
                        g2 = psp.tile([128, 4, K], F32, tag="g2")
                        for q in range(4):
                            t_local = gg * 8 + pg * 4 + q  # tile in supertile
                            sl = slice(t_local * 128, (t_local + 1) * 128)
                            # start=True zeroes the whole 2KB PSUM bank (2
                            # tiles); only the first matmul per bank sets it.
                            # PE runs in emission order, so q, q+1 are safe.
                            nc.tensor.matmul(
                                g2[:, q, :],
                                xh_s[:, sl],
                                ch_t[:],
                                start=(q % 2 == 0),
                                stop=(dve_copy and q == 3),
                                skip_group_check=True,
                            )
                        if dve_copy:
                            # DVE: h16 = g2 - (csq - 128), csq folded here
                            csqb = csqs_t[:].unsqueeze(1).broadcast_to([128, 4, K])
                            nc.vector.tensor_tensor(
                                out=h16[:, pg * 4 : pg * 4 + 4, :],
                                in0=g2[:],
                                in1=csqb,
                                op=mybir.AluOpType.subtract,
                            )
                        else:
                            # csq fold on PE: adds (128 - csq), 2 tiles per
                            # matmul (PSUM bank limit: output <= 2 KB/bank)
                            nc.tensor.matmul(
                                g2[:, 0:2, :],
                                ones_t[:],
                                csqm2_t[:],
                                start=False,
                                stop=False,
                                skip_group_check=True,
                            )
                            nc.tensor.matmul(
                                g2[:, 2:4, :],
                                ones_t[:],
                                csqm2_t[:],
                                start=False,
                                stop=True,
                                skip_group_check=True,
                            )
                            nc.scalar.activation(
                                out=h16[:, pg * 4 : pg * 4 + 4, :],
                                in_=g2[:],
                                func=mybir.ActivationFunctionType.Copy,
                            )
                    # hierarchical max over K within the 8-tile group
                    m1 = mxp.tile([128, 8, K // 2], F16, tag="m1")
                    m2 = mxp.tile([128, 8, K // 4], F16, tag="m2")
                    l1_eng = (
                        nc.gpsimd
                        if g % pool_l1_every == pool_l1_every - 1
                        else nc.vector
                    )
                    l1_eng.tensor_tensor(
                        out=m1[:],
                        in0=h16[:, :, 0 : K // 2],
                        in1=h16[:, :, K // 2 : K],
                        op=mybir.AluOpType.max,
                    )
                    nc.vector.tensor_tensor(
                        out=m2[:],
                        in0=m1[:, :, 0 : K // 4],
                        in1=m1[:, :, K // 4 : K // 2],
                        op=mybir.AluOpType.max,
                    )
                    nc.vector.tensor_reduce(
                        hmax_acc[:, g * 8 : g * 8 + 8],
                        m2[:],
                        axis=mybir.AxisListType.X,
                        op=mybir.AluOpType.max,
                    )
                    # onehot, tile-routed between DVE (1 in dve_every) and Pool;
                    # last groups split across both engines to drain fast
                    n_groups = ntiles // 8
                    oh = ohp.tile([128, 8, K], F8, tag="oh")

                    def oh_dve(q0, q1):
                        for q in range(q0, q1):
                            j = g * 8 + q
                            nc.vector.tensor_scalar(
                                out=oh[:, q, :],
                                in0=h16[:, q, :],
                                scalar1=hmax_acc[:, j : j + 1],
                                scalar2=None,
                                op0=mybir.AluOpType.is_ge,
                            )

                    def oh_pool(q0, q1):
                        # walrus rejects TensorTensor on Pool; per-tile
                        # tensor_scalar (TensorScalarPtr) is the valid form
                        for q in range(q0, q1):
                            j = g * 8 + q
                            nc.gpsimd.tensor_scalar(
                                out=oh[:, q, :],
                                in0=h16[:, q, :],
                                scalar1=hmax_acc[:, j : j + 1],
                                scalar2=None,
                                op0=mybir.AluOpType.is_ge,
                            )

                    if g >= n_groups - 2:
                        oh_pool(0, 4)
                        oh_dve(4, 8)
                    elif g % dve_every == dve_every - 1:
                        oh_dve(0, 8)
                    else:
                        # steady state: nd4/4 tiles per group on DVE, rest Pool
                        n_d = nd4 // 4 + (1 if g % 4 < nd4 % 4 else 0)
                        oh_dve(0, n_d)
                        oh_pool(n_d, 8)
                    # conf accumulation deferred by one group so the in-order
                    # PE queue never stalls waiting for this group's onehot
                    pending_conf.append(emit_conf(g, oh))
                    if len(pending_conf) > conf_defer:
                        pending_conf.pop(0)()
                    # drain hmax output as it completes (Pool queue: 25 ns
                    # dispatch vs 500+ on SP, matters for the last chunk)
                    if g % 8 == 7:
                        nc.gpsimd.dma_start(
                            hmax_out[:, (g - 7) * 8 : (g + 1) * 8],
                            hmax_acc[:, (g - 7) * 8 : (g + 1) * 8],
                        )
                col0 += width

            for emit in pending_conf:
                emit()

            conf_sb = accp.tile([NGCP, K], F32, tag="confsb")
            nc.vector.tensor_copy(conf_sb[:], conf_ps[:])
            nc.gpsimd.dma_start(conf_out[:, :], conf_sb[:])

    nc.compile()
    return nc


def kernel(x, y, centers):
    x = np.asarray(x, dtype=np.float32)
    y_np = np.asarray(y).astype(np.int64)
    centers = np.asarray(centers, dtype=np.float32)
    n = x.shape[0]
    assert n == N_FULL and x.shape[1] == D and centers.shape == (K, D)

    if "nc" not in _CACHE:
        _CACHE["nc"] = build_nc()
    nc = _CACHE["nc"]

    f8np = mybir.dt.np(F8)

    xt = np.ascontiguousarray(x.T).astype(np.float16)  # [128, N]
    ch = np.ascontiguousarray(centers.T * np.float32(2.0)).astype(np.float16)
    csq = np.sum(centers.astype(np.float64) ** 2, axis=1)
    csqm = (np.float64(OFF) - csq).astype(np.float16)  # [K]
    csqm2 = np.ascontiguousarray(np.tile(csqm, 2)[None, :])  # [1, 2K]
    csqs32 = np.ascontiguousarray(
        np.broadcast_to((csq - np.float64(OFF)).astype(np.float32)[None, :], (128, K))
    )

    # yoh pairs: [core, 128, npairs, 2, 10] in fp8
    y_cores = y_np.reshape(NUM_CORES, NTILES, 128)
    oh = y_cores[:, :, :, None] == np.arange(NGCP)[None, None, None, :]
    # [core, tile, p, c] -> [core, p(128), pair, j, c]  (classes 10..15 never
    # match y, so the padded conf rows stay zero)
    oh = oh.reshape(NUM_CORES, NTILES // 2, 2, 128, NGCP).transpose(0, 3, 1, 2, 4)
    yohp_all = np.ascontiguousarray(
        oh.reshape(NUM_CORES, 128, (NTILES // 2) * 2 * NGCP)
    ).astype(f8np)

    in_maps = []
    for c in range(NUM_CORES):
        sl = slice(c * NS, (c + 1) * NS)
        in_maps.append(
            {
                "xh": np.ascontiguousarray(xt[:, sl]),
                "ch": ch,
                "csqm2": csqm2,
                "csqs32": csqs32,
                "yohp": yohp_all[c],
            }
        )

    full_res = run_bass_kernel_spmd(nc, in_maps, list(range(NUM_CORES)))
    _CACHE["last_results"] = full_res
    res = full_res.results

    hmax_sum = 0.0
    conf = np.zeros((K, NGC), dtype=np.float64)
    for c in range(NUM_CORES):
        hmax_sum += float(np.asarray(res[c]["hmax"]).astype(np.float64).sum())
        conf += np.asarray(res[c]["conf"]).astype(np.float64).T[:, :NGC]  # [K, 10]

    x64 = x.astype(np.float64)
    x_sq_total = float(np.einsum("nd,nd->", x64, x64, optimize=True))
    loss = np.float32(x_sq_total + OFF * n - hmax_sum)

    correct_ct = conf.max(axis=1).sum()
    acc = np.float32(correct_ct / np.float32(n))
    return loss, acc
